# revision 46
# baseline (speedup 1.0000x reference)
"""Additive attention kernel for Trainium2, 8 NeuronCores (SPMD).

Reference computation (B=4, Q=128, K=1024, H=256, QS=KS=DV=256):
    q = queries @ Wq                    [B,Q,H]
    k = keys @ Wk                       [B,K,H]
    feats = tanh(q[:,:,None,:] + k[:,None,:,:])
    scores = feats @ Wv                 [B,Q,K]
    masked softmax over K (valid_lens), out = attn @ values   [B,Q,DV]

Sharding: Q is split across the 8 cores (16 q-rows per core, every core
processes all 4 batches) -- perfectly balanced, no collectives.  The
kernel is specialized at trace time on the runtime valid_lens values, so
only valid key positions are ever computed.

Per-core device program (H on partitions, 2 chunks of 128).  The ACT
(scalar) engine's tanh stream is the roofline (1 elem/cycle/lane); the
whole program is organized to keep it saturated:

  DMA: the packed bf16 blob is split into ~10 column segments spread over
    the three DMA-capable queues (sync HWDGE, scalar HWDGE, gpsimd SWDGE),
    ordered by consumer need-time so the first batch's projection inputs
    land ~1.5us after kickoff and everything else streams in under the
    tanh phases.  Batches are processed smallest-first (fast pipeline
    ramp); the last batch is chosen small to shorten the drain tail.
  Projections (PE, bf16, fp32 PSUM accumulate over the 2 e-chunks):
    qprojT[hc] = Wq_hc.T @ qT [128, B*16]; kprojT per batch [128, VLP].
    kproj for batch b+1 is emitted in the middle of batch b's sweep so PE
    fills its idle slots; all PSUM->SBUF kp casts live on the DVE (the
    ACT engine runs nothing but tanh+exp).
  Scores, per (b, hc, slab of q's): DVE tensor_scalar_add (kprojT + qp
    column, 4x mode) into a bf16 slab, ONE ACT Tanh per slab, then per q
    a matmul whose lhsT is a width-32 slice of a sliding one-hot Wv tile,
    round-robined over the four PE column groups (tile_position=(0,32*s),
    s=q%4, one-hot at row q//4).  The score PSUM banks are zeroed by DVE
    memsets issued up-front during the DMA wait (zero marginal cost).
    Only the very first matmul per PSUM bank uses start=True (the start
    flag clears has_written for the WHOLE bank; concurrent per-stripe
    starts race).
  Softmax + AV, per batch: exp straight off the PSUM stripes (ACT, bf16
    out; zero rows give exp=1, never read; |scores| <= sum|Wv| ~ 13 so
    no max shift), PE bf16 transpose per 128-chunk, DVE compaction of
    the 16 live columns, AV matmul against the ones-augmented bf16
    values (the ones column accumulates the softmax denominator), then
    the [16, DV+1] AV accumulator ships straight from PSUM to DRAM; the
    softmax division and the q-permutation introduced by the striping
    (q -> 4*(q%4) + q//4) are undone on the host after gather.
"""

import numpy as np

B, Q, KMAX, H = 4, 128, 1024, 256
QS, KS, DV = 256, 256, 256
N_CORES = 8
QSH = Q // N_CORES  # 16 q rows per core

# q index permutation induced by the 4-way PSUM striping (self-inverse)
PERM = np.array([4 * (c % 4) + c // 4 for c in range(QSH)])

_PROGRAM_CACHE: dict = {}


def _even(x: int) -> int:
    return x + (x & 1)


def _windows(total: int, step: int):
    out = []
    s = 0
    while s < total:
        out.append((s, min(step, total - s)))
        s += step
    return out


def _batch_order(VL):
    """Second-largest batch first (its long, ACT-bound tanh slabs absorb
    the DVE ramp: qp copies + next batch's kp casts), then the largest,
    then descending, smallest last (short drain tail)."""
    order = sorted(range(B), key=lambda b: -VL[b])
    return [order[1], order[0]] + order[2:]


def _slabs(b, b_idx, hc, n_batches, VLP):
    """Slab sizes (q's per tanh) for batch b at position b_idx."""
    if b_idx == 0 and hc == 0:
        return [4, 4, 8]  # ramp-in: first tanh fires after only 4 adds
    if b_idx == n_batches - 1 and hc == 1:
        return [8, 4, 4]  # fine-grained drain
    if b_idx == 1 and hc == 0:
        # batch-to-batch entry: the first tanh here only needs 4 adds
        # pre-buffered during the previous batch's final slab
        return [4, 4, 8]
    if hc == 1 and 0 < b_idx < n_batches - 1:
        # mid-stream second chunk: the DVE conveyor is far enough ahead
        # that one big ACTIVATE (half the fill overhead) is safe
        return [16]
    return [8, 8]


def _geom(vl):
    """Shared geometry: valid lens, paddings, blob column offsets.

    Blob columns are laid out in DMA-segment order: each segment is a
    contiguous column range pushed as one dma_start on its queue.
    """
    VL = list(vl)
    VLP = [_even(v) for v in VL]
    NCH = [(v + 127) // 128 for v in VL]
    BORD = _batch_order(VL)

    WB = {b: _windows(VLP[b], 512) for b in range(B)}
    KW = []  # (global window idx) -> (w0-in-batch, wl, batch)
    for b in range(B):
        for (u0, ul) in WB[b]:
            KW.append((u0, ul, b))

    cols = {}
    segs = {"sync": [], "scalar": [], "gpsimd": []}
    c = 0

    def seg(queue, names_widths):
        nonlocal c
        c0 = c
        for name, w in names_widths:
            cols[name] = c
            c += w
        segs[queue].append((c0, c))

    def keys_seg(b):
        out = []
        for wi, (_w0, wl, wb) in enumerate(KW):
            if wb == b:
                for e in range(2):
                    out.append((f"keysT_w{wi}_e{e}", wl))
        return out

    def vals_names(b):
        return [(f"vals{b}", NCH[b] * (DV + 1))]

    # sync queue: projections first, then values in batch order
    seg("sync", [("wq0", H), ("wq1", H), ("qT0", B * QSH), ("qT1", B * QSH)])
    seg("sync", vals_names(BORD[0]) + [("ident", 128)])
    seg("sync", vals_names(BORD[2]) + vals_names(BORD[3]))
    # scalar queue: first batch's kproj inputs, then second batch's keys
    seg("scalar", [("wk0", H), ("wk1", H)] + keys_seg(BORD[0]))
    seg("scalar", keys_seg(BORD[1]))
    # gpsimd queue: one-hot Wv tiles, remaining keys, 2nd batch's values
    seg("gpsimd", [("wz0", QSH + 31), ("wz1", QSH + 31)])
    seg("gpsimd", keys_seg(BORD[2]))
    seg("gpsimd", keys_seg(BORD[3]))
    seg("gpsimd", vals_names(BORD[1]))
    cols["zpad"] = c
    c += 16
    CW = c
    return VL, VLP, NCH, BORD, WB, KW, cols, segs, CW


def _build_program(vl: tuple):
    import concourse.bacc as bacc
    import concourse.mybir as mybir
    import concourse.tile as tile

    dt = mybir.dt
    AF = mybir.ActivationFunctionType
    VL, VLP, NCH, BORD, WB, KW, cols, segs, CW = _geom(vl)

    nc = bacc.Bacc("TRN2", target_bir_lowering=False, debug=False,
                   num_devices=N_CORES)

    d_blob = nc.dram_tensor("blob", [128, CW], dt.bfloat16,
                            kind="ExternalInput")
    d_out = nc.dram_tensor("out", [B, QSH, DV + 1], dt.bfloat16,
                           kind="ExternalOutput")

    with tile.TileContext(nc) as tc:
        with (
            tc.tile_pool(name="const", bufs=1) as constp,
            tc.tile_pool(name="kp", bufs=1) as kpp,
            tc.tile_pool(name="qp", bufs=2) as qpp,
            tc.tile_pool(name="pre", bufs=5) as prep,
            tc.tile_pool(name="feats", bufs=5) as featsp,
            tc.tile_pool(name="scsb", bufs=3) as scsbp,
            tc.tile_pool(name="expt", bufs=4) as exptp,
            tc.tile_pool(name="outsb", bufs=2) as outsbp,
            tc.tile_pool(name="pswork", bufs=2, space="PSUM") as pswork,
            tc.tile_pool(name="pssc", bufs=5, space="PSUM") as pssc,
            tc.tile_pool(name="psav", bufs=1, space="PSUM") as psav,
        ):
            blob = constp.tile([128, CW], dt.bfloat16, tag="blob")
            eng_of = {"sync": nc.sync, "scalar": nc.scalar,
                      "gpsimd": nc.gpsimd}
            for qname in ("sync", "scalar", "gpsimd"):
                for (c0, c1) in segs[qname]:
                    eng_of[qname].dma_start(out=blob[:, c0:c1],
                                            in_=d_blob[:, c0:c1])

            def bl(name, width):
                c0 = cols[name]
                return blob[:, c0:c0 + width]

            # ---- q projection ----
            qp = [None, None]  # [hc] -> [128, B*QSH] f32 in SBUF
            for hc in range(2):
                hs = slice(hc * 128, hc * 128 + 128)
                ps = pswork.tile([128, B * QSH], dt.float32, tag="w",
                                 name=f"psq{hc}")
                nc.tensor.matmul(ps[:], bl("wq0", H)[:, hs],
                                 bl("qT0", B * QSH), start=True, stop=False)
                nc.tensor.matmul(ps[:], bl("wq1", H)[:, hs],
                                 bl("qT1", B * QSH), start=False, stop=True)
                qp[hc] = qpp.tile([128, B * QSH], dt.float32, tag="qp",
                                  name=f"qp{hc}")
                nc.vector.tensor_copy(qp[hc][:], ps[:])

            # ---- k projection (per batch, emitted lazily) ----
            # PE matmuls and DVE casts are emitted separately: the PE has
            # slack early (it idles until the first tanh lands), but a
            # cast emitted too early blocks the DVE's add stream behind
            # the next batch's keysT DMA.
            kp = {}       # b -> [hc] -> [128, VLP[b]] bf16 in SBUF
            kp_ps = {}    # b -> list of (hc, w0, wl, psum tile)

            def emit_kproj_mm(b):
                kp[b] = [kpp.tile([128, VLP[b]], dt.bfloat16,
                                  tag=f"kp{b}_{hc}", name=f"kp{b}_{hc}")
                         for hc in range(2)]
                kp_ps[b] = []
                for wi, (w0, wl, wb) in enumerate(KW):
                    if wb != b:
                        continue
                    for hc in range(2):
                        hs = slice(hc * 128, hc * 128 + 128)
                        ps2 = pswork.tile([128, wl], dt.float32, tag="w",
                                          name=f"psk{hc}_{b}_{w0}")
                        nc.tensor.matmul(ps2[:], bl("wk0", H)[:, hs],
                                         bl(f"keysT_w{wi}_e0", wl),
                                         start=True, stop=False)
                        nc.tensor.matmul(ps2[:], bl("wk1", H)[:, hs],
                                         bl(f"keysT_w{wi}_e1", wl),
                                         start=False, stop=True)
                        kp_ps[b].append((hc, w0, wl, ps2))

            def emit_kproj_cast(b, hcs=(0, 1)):
                for hc, w0, wl, ps2 in kp_ps[b]:
                    if hc in hcs:
                        nc.vector.tensor_copy(
                            kp[b][hc][:, w0:w0 + wl], ps2[:])

            emit_kproj_mm(BORD[0])
            emit_kproj_cast(BORD[0])
            emit_kproj_mm(BORD[1])

            # score PSUM banks: allocate + zero all of them up-front, on
            # the DVE, while it would otherwise idle waiting for DMA
            sc_tiles = {}
            for b in BORD:
                for wi in range(len(WB[b])):
                    t = pssc.tile([128, 512], dt.float32, tag="sc",
                                  name=f"sc{b}_{wi}")
                    nc.vector.memset(t[:], 0.0)
                    sc_tiles[b, wi] = t

            for bi, b in enumerate(BORD):
                vlb, vlpb = VL[b], VLP[b]
                # ---- scores sweep for batch b (4-way col-packed) ----
                sc_ps = [sc_tiles[b, wi] for wi in range(len(WB[b]))]
                for hc in range(2):
                    q0 = 0
                    for gi, gsz in enumerate(_slabs(b, bi, hc, B, VLP)):
                        pre_t = prep.tile([128, gsz * vlpb], dt.bfloat16,
                                          tag="pre", name=f"pre{b}{hc}{gi}")
                        for j in range(gsz):
                            qq = q0 + j
                            nc.vector.tensor_scalar_add(
                                pre_t[:, j * vlpb:(j + 1) * vlpb],
                                kp[b][hc][:],
                                qp[hc][:, b * QSH + qq:b * QSH + qq + 1])
                        feats_t = featsp.tile([128, gsz * vlpb], dt.bfloat16,
                                              tag="feats",
                                              name=f"ft{b}{hc}{gi}")
                        nc.scalar.activation(feats_t[:], pre_t[:], AF.Tanh)
                        if bi == 0 and gi == len(
                                _slabs(b, bi, hc, B, VLP)) - 1:
                            # DVE slack opens before the last slab of each
                            # hc phase: pull the next batch's kp casts in,
                            # half per phase so neither phase bubbles
                            emit_kproj_cast(BORD[1], hcs=(hc,))
                        for j in range(gsz):
                            qq = q0 + j
                            s, r = qq % 4, qq // 4
                            # one-hot column r within a width-32 window
                            lhsT = bl(f"wz{hc}", QSH + 31)[
                                :, QSH - 1 - r: QSH + 31 - r]
                            first = (hc == 0 and qq == 0)
                            last = (hc == 1 and qq == QSH - 1)
                            for wi, (w0, wl) in enumerate(WB[b]):
                                nc.tensor.matmul(
                                    sc_ps[wi][32 * s:32 * s + 32, 0:wl],
                                    lhsT,
                                    feats_t[:, j * vlpb + w0:
                                            j * vlpb + w0 + wl],
                                    start=first, stop=last,
                                    tile_position=(0, 32 * s),
                                    skip_group_check=True)
                        q0 += gsz
                    if hc == 0:
                        # keep the pipeline fed: batch bi+2's kproj MMs
                        # and batch bi+1's deferred kp casts go out here
                        if bi >= 1 and bi + 1 < B:
                            emit_kproj_cast(BORD[bi + 1])
                        if bi + 2 < B:
                            emit_kproj_mm(BORD[bi + 2])

                # ---- scores -> softmax -> AV for batch b ----
                # exp straight off the PSUM stripes (garbage-free rows are
                # exactly 0 -> exp=1, never read), then bf16 transpose
                exp_sb = scsbp.tile([128, vlb], dt.bfloat16, tag="scsb",
                                    name=f"scsb{b}")
                for wi, (w0, wl) in enumerate(WB[b]):
                    wle = min(wl, vlb - w0)  # drop the pad column
                    nc.scalar.activation(exp_sb[:, w0:w0 + wle],
                                         sc_ps[wi][:, 0:wle], AF.Exp)

                av = psav.tile([QSH, DV + 1], dt.float32, tag="av",
                               name=f"av{b}")
                for ci in range(NCH[b]):
                    c0 = ci * 128
                    csz = min(128, vlb - c0)
                    trp = pswork.tile([csz, 128], dt.bfloat16, tag="w",
                                      name=f"tr{b}_{ci}")
                    nc.tensor.transpose(trp[:], exp_sb[:, c0:c0 + csz],
                                        bl("ident", 128))
                    ex = exptp.tile([csz, QSH], dt.bfloat16, tag="ex",
                                    name=f"ex{b}_{ci}")
                    nc.vector.tensor_copy(
                        ex[:].rearrange("p (s r) -> p s r", s=4),
                        trp[:].rearrange("p (s x r) -> p s x r",
                                         s=4, x=8)[:, :, 0, 0:4])
                    vcol = cols[f"vals{b}"] + ci * (DV + 1)
                    nc.tensor.matmul(av[:], ex[:],
                                     blob[0:csz, vcol:vcol + DV + 1],
                                     start=(ci == 0),
                                     stop=(ci == NCH[b] - 1))

                # ship numerator + denominator; the host divides after
                # gather.  Mid-stream batches go on the (idle) gpsimd
                # queue; the final batch uses the sync HWDGE queue, whose
                # hardware descriptor generation shaves ~1us off the
                # critical out-DMA -> final-barrier path.
                ob = outsbp.tile([QSH, DV + 1], dt.bfloat16, tag="ob",
                                 name=f"ob{b}")
                nc.vector.tensor_copy(ob[:], av[:])
                eng = nc.sync if bi == B - 1 else nc.gpsimd
                eng.dma_start(out=d_out[b], in_=ob[:])

    nc.compile()
    return nc


def _host_prep(queries, keys, values, vl, Wq, Wk, Wv):
    """Build the 8 per-core input maps (slicing / transposes / packing)."""
    import ml_dtypes
    bf16 = ml_dtypes.bfloat16

    queries = np.ascontiguousarray(np.asarray(queries, np.float32))
    keys = np.asarray(keys, np.float32)
    values = np.asarray(values, np.float32)
    Wq = np.asarray(Wq, np.float32)
    Wk = np.asarray(Wk, np.float32)
    Wv = np.asarray(Wv, np.float32)

    VL, VLP, NCH, BORD, WB, KW, cols, segs, CW = _geom(vl)

    shared = np.zeros((128, CW), np.float32)
    for e in range(2):
        shared[:, cols[f"wq{e}"]:cols[f"wq{e}"] + H] = \
            Wq[e * 128:(e + 1) * 128, :]
        shared[:, cols[f"wk{e}"]:cols[f"wk{e}"] + H] = \
            Wk[e * 128:(e + 1) * 128, :]
    for wi, (w0, wl, b) in enumerate(KW):
        wle = min(wl, VL[b] - w0)
        kT = keys[b, w0:w0 + wle, :].T.reshape(2, 128, wle)
        for e in range(2):
            c0 = cols[f"keysT_w{wi}_e{e}"]
            shared[:, c0:c0 + wle] = kT[e]
    for hc in range(2):
        shared[:, cols[f"wz{hc}"] + QSH - 1] = Wv[hc * 128:(hc + 1) * 128]
    for b in range(B):
        for ci in range(NCH[b]):
            c0 = ci * 128
            csz = min(128, VL[b] - c0)
            vcol = cols[f"vals{b}"] + ci * (DV + 1)
            shared[:csz, vcol:vcol + DV] = values[b, c0:c0 + csz, :]
            shared[:csz, vcol + DV] = 1.0
    shared[:, cols["ident"]:cols["ident"] + 128] = np.eye(128)

    in_maps = []
    for c in range(N_CORES):
        blob = shared.copy()
        qsl = queries[:, c * QSH:(c + 1) * QSH, :]  # [B, 16, QS]
        qT = np.ascontiguousarray(qsl.transpose(0, 2, 1))  # [B, QS, 16]
        for e in range(2):
            c0 = cols[f"qT{e}"]
            for b in range(B):
                blob[:, c0 + b * QSH:c0 + (b + 1) * QSH] = \
                    qT[b, e * 128:(e + 1) * 128, :]
        in_maps.append({"blob": blob.astype(bf16)})
    return in_maps


def _gather(results):
    out = np.empty((B, Q, DV), np.float32)
    for c in range(N_CORES):
        # device rows are in PERM order: row c holds q = PERM[c]; the
        # last column carries the softmax denominator (divide on host)
        av = np.asarray(results[c]["out"], np.float32)
        out[:, c * QSH + PERM, :] = av[:, :, :DV] / av[:, :, DV:DV + 1]
    return out


def kernel(queries, keys, values, valid_lens, Wq, Wk, Wv):
    from concourse.bass_utils import run_bass_kernel_spmd

    vl = tuple(int(x) for x in np.asarray(valid_lens).reshape(-1))
    assert len(vl) == B and all(1 <= v <= KMAX for v in vl)

    if vl not in _PROGRAM_CACHE:
        _PROGRAM_CACHE[vl] = _build_program(vl)
    nc = _PROGRAM_CACHE[vl]

    in_maps = _host_prep(queries, keys, values, vl, Wq, Wk, Wv)
    res = run_bass_kernel_spmd(nc, in_maps, list(range(N_CORES)))
    return _gather(res.results)


# revision 48
# speedup vs baseline: 1.1706x; 1.1706x over previous
"""Additive attention kernel for Trainium2, 8 NeuronCores (SPMD).

Reference computation (B=4, Q=128, K=1024, H=256, QS=KS=DV=256):
    q = queries @ Wq                    [B,Q,H]
    k = keys @ Wk                       [B,K,H]
    feats = tanh(q[:,:,None,:] + k[:,None,:,:])
    scores = feats @ Wv                 [B,Q,K]
    masked softmax over K (valid_lens), out = attn @ values   [B,Q,DV]

Sharding: Q is split across the 8 cores (16 q-rows per core, every core
processes all 4 batches) -- perfectly balanced, no collectives.  The
kernel is specialized at trace time on the runtime valid_lens values, so
only valid key positions are ever computed.

Per-core device program (H on partitions, 2 chunks of 128).  The ACT
(scalar) engine's tanh stream is the roofline (1 elem/cycle/lane); the
whole program is organized to keep it saturated:

  DMA: the packed bf16 blob is split into ~10 column segments spread over
    the three DMA-capable queues (sync HWDGE, scalar HWDGE, gpsimd SWDGE),
    ordered by consumer need-time so the first batch's projection inputs
    land ~1.5us after kickoff and everything else streams in under the
    tanh phases.  Batches are processed smallest-first (fast pipeline
    ramp); the last batch is chosen small to shorten the drain tail.
  Projections (PE, bf16, fp32 PSUM accumulate over the 2 e-chunks):
    qprojT[hc] = Wq_hc.T @ qT [128, B*16]; kprojT per batch [128, VLP].
    kproj for batch b+1 is emitted in the middle of batch b's sweep so PE
    fills its idle slots; all PSUM->SBUF kp casts live on the DVE (the
    ACT engine runs nothing but tanh+exp).
  Scores, per (b, hc, slab of q's): DVE tensor_scalar_add (kprojT + qp
    column, 4x mode) into a bf16 slab, ONE ACT Tanh per slab, then per q
    a matmul whose lhsT is a width-32 slice of a sliding one-hot Wv tile,
    round-robined over the four PE column groups (tile_position=(0,32*s),
    s=q%4, one-hot at row q//4).  The score PSUM banks are zeroed by DVE
    memsets issued up-front during the DMA wait (zero marginal cost).
    Only the very first matmul per PSUM bank uses start=True (the start
    flag clears has_written for the WHOLE bank; concurrent per-stripe
    starts race).
  Softmax + AV, per batch: exp straight off the PSUM stripes (ACT, bf16
    out; zero rows give exp=1, never read; |scores| <= sum|Wv| ~ 13 so
    no max shift), PE bf16 transpose per 128-chunk, DVE compaction of
    the 16 live columns, AV matmul against the ones-augmented bf16
    values (the ones column accumulates the softmax denominator), then
    the [16, DV+1] AV accumulator ships straight from PSUM to DRAM; the
    softmax division and the q-permutation introduced by the striping
    (q -> 4*(q%4) + q//4) are undone on the host after gather.
"""

import numpy as np

B, Q, KMAX, H = 4, 128, 1024, 256
QS, KS, DV = 256, 256, 256
N_CORES = 8
QSH = Q // N_CORES  # 16 q rows per core

# q index permutation induced by the 4-way PSUM striping (self-inverse)
PERM = np.array([4 * (c % 4) + c // 4 for c in range(QSH)])

_PROGRAM_CACHE: dict = {}


def _even(x: int) -> int:
    return x + (x & 1)


def _windows(total: int, step: int):
    out = []
    s = 0
    while s < total:
        out.append((s, min(step, total - s)))
        s += step
    return out


def _batch_order(VL):
    """Second-largest batch first (its long, ACT-bound tanh slabs absorb
    the DVE ramp: qp copies + next batch's kp casts), then the largest,
    then descending, smallest last (short drain tail)."""
    order = sorted(range(B), key=lambda b: -VL[b])
    return [order[1], order[0]] + order[2:]


def _slabs(b, b_idx, hc, n_batches, VLP):
    """Slab sizes (q's per tanh) for batch b at position b_idx."""
    if b_idx == 0 and hc == 0:
        return [4, 4, 8]  # ramp-in: first tanh fires after only 4 adds
    if b_idx == n_batches - 1 and hc == 1:
        return [8, 4, 4]  # fine-grained drain
    if b_idx == 1 and hc == 0:
        # batch-to-batch entry: the first tanh here only needs 4 adds
        # pre-buffered during the previous batch's final slab
        return [4, 4, 8]
    if hc == 1 and 0 < b_idx < n_batches - 1:
        # mid-stream second chunk: the DVE conveyor is far enough ahead
        # that one big ACTIVATE (half the fill overhead) is safe
        return [16]
    return [8, 8]


def _geom(vl):
    """Shared geometry: valid lens, paddings, blob column offsets.

    Blob columns are laid out in DMA-segment order: each segment is a
    contiguous column range pushed as one dma_start on its queue.
    """
    VL = list(vl)
    VLP = [_even(v) for v in VL]
    NCH = [(v + 127) // 128 for v in VL]
    BORD = _batch_order(VL)

    WB = {b: _windows(VLP[b], 512) for b in range(B)}
    KW = []  # (global window idx) -> (w0-in-batch, wl, batch)
    for b in range(B):
        for (u0, ul) in WB[b]:
            KW.append((u0, ul, b))

    cols = {}
    segs = {"sync": [], "scalar": [], "gpsimd": []}
    c = 0

    def seg(queue, names_widths):
        nonlocal c
        c0 = c
        for name, w in names_widths:
            cols[name] = c
            c += w
        segs[queue].append((c0, c))

    def keys_seg(b):
        out = []
        for wi, (_w0, wl, wb) in enumerate(KW):
            if wb == b:
                for e in range(2):
                    out.append((f"keysT_w{wi}_e{e}", wl))
        return out

    def vals_names(b):
        return [(f"vals{b}", NCH[b] * (DV + 1))]

    # sync queue: projections first, then values in batch order
    seg("sync", [("wq0", H), ("wq1", H), ("qT0", B * QSH), ("qT1", B * QSH)])
    seg("sync", vals_names(BORD[0]) + [("ident", 128)])
    seg("sync", vals_names(BORD[2]) + vals_names(BORD[3]))
    # scalar queue: first batch's kproj inputs, then second batch's keys
    seg("scalar", [("wk0", H), ("wk1", H)] + keys_seg(BORD[0]))
    seg("scalar", keys_seg(BORD[1]))
    # gpsimd queue: one-hot Wv tiles, remaining keys, 2nd batch's values
    seg("gpsimd", [("wz0", QSH + 31), ("wz1", QSH + 31)])
    seg("gpsimd", keys_seg(BORD[2]))
    seg("gpsimd", keys_seg(BORD[3]))
    seg("gpsimd", vals_names(BORD[1]))
    cols["zpad"] = c
    c += 16
    CW = c
    return VL, VLP, NCH, BORD, WB, KW, cols, segs, CW


def _build_program(vl: tuple):
    import concourse.bacc as bacc
    import concourse.mybir as mybir
    import concourse.tile as tile

    dt = mybir.dt
    AF = mybir.ActivationFunctionType
    VL, VLP, NCH, BORD, WB, KW, cols, segs, CW = _geom(vl)

    nc = bacc.Bacc("TRN2", target_bir_lowering=False, debug=False,
                   num_devices=N_CORES)

    d_blob = nc.dram_tensor("blob", [128, CW], dt.bfloat16,
                            kind="ExternalInput")
    d_out = nc.dram_tensor("out", [B, QSH, DV + 1], dt.bfloat16,
                           kind="ExternalOutput")

    with tile.TileContext(nc) as tc:
        with (
            tc.tile_pool(name="const", bufs=1) as constp,
            tc.tile_pool(name="kp", bufs=1) as kpp,
            tc.tile_pool(name="qp", bufs=2) as qpp,
            tc.tile_pool(name="pre", bufs=5) as prep,
            tc.tile_pool(name="feats", bufs=5) as featsp,
            tc.tile_pool(name="scsb", bufs=3) as scsbp,
            tc.tile_pool(name="expt", bufs=4) as exptp,
            tc.tile_pool(name="outsb", bufs=2) as outsbp,
            tc.tile_pool(name="pswork", bufs=2, space="PSUM") as pswork,
            tc.tile_pool(name="pssc", bufs=5, space="PSUM") as pssc,
            tc.tile_pool(name="psav", bufs=1, space="PSUM") as psav,
        ):
            blob = constp.tile([128, CW], dt.bfloat16, tag="blob")
            eng_of = {"sync": nc.sync, "scalar": nc.scalar,
                      "gpsimd": nc.gpsimd}
            for qname in ("sync", "scalar", "gpsimd"):
                for (c0, c1) in segs[qname]:
                    eng_of[qname].dma_start(out=blob[:, c0:c1],
                                            in_=d_blob[:, c0:c1])

            def bl(name, width):
                c0 = cols[name]
                return blob[:, c0:c0 + width]

            # ---- q projection ----
            qp = [None, None]  # [hc] -> [128, B*QSH] f32 in SBUF
            for hc in range(2):
                hs = slice(hc * 128, hc * 128 + 128)
                ps = pswork.tile([128, B * QSH], dt.float32, tag="w",
                                 name=f"psq{hc}")
                nc.tensor.matmul(ps[:], bl("wq0", H)[:, hs],
                                 bl("qT0", B * QSH), start=True, stop=False)
                nc.tensor.matmul(ps[:], bl("wq1", H)[:, hs],
                                 bl("qT1", B * QSH), start=False, stop=True)
                qp[hc] = qpp.tile([128, B * QSH], dt.float32, tag="qp",
                                  name=f"qp{hc}")
                nc.vector.tensor_copy(qp[hc][:], ps[:])

            # ---- k projection (per batch, emitted lazily) ----
            # PE matmuls and DVE casts are emitted separately: the PE has
            # slack early (it idles until the first tanh lands), but a
            # cast emitted too early blocks the DVE's add stream behind
            # the next batch's keysT DMA.
            kp = {}       # b -> [hc] -> [128, VLP[b]] bf16 in SBUF
            kp_ps = {}    # b -> list of (hc, w0, wl, psum tile)

            def emit_kproj_mm(b):
                kp[b] = [kpp.tile([128, VLP[b]], dt.bfloat16,
                                  tag=f"kp{b}_{hc}", name=f"kp{b}_{hc}")
                         for hc in range(2)]
                kp_ps[b] = []
                for wi, (w0, wl, wb) in enumerate(KW):
                    if wb != b:
                        continue
                    for hc in range(2):
                        hs = slice(hc * 128, hc * 128 + 128)
                        ps2 = pswork.tile([128, wl], dt.float32, tag="w",
                                          name=f"psk{hc}_{b}_{w0}")
                        nc.tensor.matmul(ps2[:], bl("wk0", H)[:, hs],
                                         bl(f"keysT_w{wi}_e0", wl),
                                         start=True, stop=False)
                        nc.tensor.matmul(ps2[:], bl("wk1", H)[:, hs],
                                         bl(f"keysT_w{wi}_e1", wl),
                                         start=False, stop=True)
                        kp_ps[b].append((hc, w0, wl, ps2))

            def emit_kproj_cast(b, hcs=(0, 1)):
                for hc, w0, wl, ps2 in kp_ps[b]:
                    if hc in hcs:
                        nc.vector.tensor_copy(
                            kp[b][hc][:, w0:w0 + wl], ps2[:])

            emit_kproj_mm(BORD[0])
            emit_kproj_cast(BORD[0])
            emit_kproj_mm(BORD[1])

            # score PSUM banks: allocate + zero all of them up-front, on
            # the DVE, while it would otherwise idle waiting for DMA
            sc_tiles = {}
            for b in BORD:
                for wi in range(len(WB[b])):
                    t = pssc.tile([128, 512], dt.float32, tag="sc",
                                  name=f"sc{b}_{wi}")
                    nc.vector.memset(t[:], 0.0)
                    sc_tiles[b, wi] = t

            for bi, b in enumerate(BORD):
                vlb, vlpb = VL[b], VLP[b]
                # ---- scores sweep for batch b (4-way col-packed) ----
                sc_ps = [sc_tiles[b, wi] for wi in range(len(WB[b]))]
                for hc in range(2):
                    q0 = 0
                    for gi, gsz in enumerate(_slabs(b, bi, hc, B, VLP)):
                        pre_t = prep.tile([128, gsz * vlpb], dt.bfloat16,
                                          tag="pre", name=f"pre{b}{hc}{gi}")
                        for j in range(gsz):
                            qq = q0 + j
                            nc.vector.tensor_scalar_add(
                                pre_t[:, j * vlpb:(j + 1) * vlpb],
                                kp[b][hc][:],
                                qp[hc][:, b * QSH + qq:b * QSH + qq + 1])
                        feats_t = featsp.tile([128, gsz * vlpb], dt.bfloat16,
                                              tag="feats",
                                              name=f"ft{b}{hc}{gi}")
                        nc.scalar.activation(feats_t[:], pre_t[:], AF.Tanh)
                        if bi == 0 and gi == len(
                                _slabs(b, bi, hc, B, VLP)) - 1:
                            # DVE slack opens before the last slab of each
                            # hc phase: pull the next batch's kp casts in,
                            # half per phase so neither phase bubbles
                            emit_kproj_cast(BORD[1], hcs=(hc,))
                        for j in range(gsz):
                            qq = q0 + j
                            s, r = qq % 4, qq // 4
                            # one-hot column r within a width-32 window
                            lhsT = bl(f"wz{hc}", QSH + 31)[
                                :, QSH - 1 - r: QSH + 31 - r]
                            first = (hc == 0 and qq == 0)
                            last = (hc == 1 and qq == QSH - 1)
                            for wi, (w0, wl) in enumerate(WB[b]):
                                nc.tensor.matmul(
                                    sc_ps[wi][32 * s:32 * s + 32, 0:wl],
                                    lhsT,
                                    feats_t[:, j * vlpb + w0:
                                            j * vlpb + w0 + wl],
                                    start=first, stop=last,
                                    tile_position=(0, 32 * s),
                                    skip_group_check=True)
                        q0 += gsz
                    if hc == 0:
                        # keep the pipeline fed: batch bi+2's kproj MMs
                        # and batch bi+1's deferred kp casts go out here
                        if bi >= 1 and bi + 1 < B:
                            emit_kproj_cast(BORD[bi + 1])
                        if bi + 2 < B:
                            emit_kproj_mm(BORD[bi + 2])

                # ---- scores -> softmax -> AV for batch b ----
                # exp straight off the PSUM stripes (garbage-free rows are
                # exactly 0 -> exp=1, never read), then bf16 transpose
                exp_sb = scsbp.tile([128, vlb], dt.bfloat16, tag="scsb",
                                    name=f"scsb{b}")
                for wi, (w0, wl) in enumerate(WB[b]):
                    wle = min(wl, vlb - w0)  # drop the pad column
                    nc.scalar.activation(exp_sb[:, w0:w0 + wle],
                                         sc_ps[wi][:, 0:wle], AF.Exp)

                av = psav.tile([QSH, DV + 1], dt.float32, tag="av",
                               name=f"av{b}")
                for ci in range(NCH[b]):
                    c0 = ci * 128
                    csz = min(128, vlb - c0)
                    trp = pswork.tile([csz, 128], dt.bfloat16, tag="w",
                                      name=f"tr{b}_{ci}")
                    nc.tensor.transpose(trp[:], exp_sb[:, c0:c0 + csz],
                                        bl("ident", 128))
                    ex = exptp.tile([csz, QSH], dt.bfloat16, tag="ex",
                                    name=f"ex{b}_{ci}")
                    nc.vector.tensor_copy(
                        ex[:].rearrange("p (s r) -> p s r", s=4),
                        trp[:].rearrange("p (s x r) -> p s x r",
                                         s=4, x=8)[:, :, 0, 0:4])
                    vcol = cols[f"vals{b}"] + ci * (DV + 1)
                    nc.tensor.matmul(av[:], ex[:],
                                     blob[0:csz, vcol:vcol + DV + 1],
                                     start=(ci == 0),
                                     stop=(ci == NCH[b] - 1))

                # ship numerator + denominator; the host divides after
                # gather.  Mid-stream batches go on the (idle) gpsimd
                # queue; the final batch uses the sync HWDGE queue, whose
                # hardware descriptor generation shaves ~1us off the
                # critical out-DMA -> final-barrier path.
                ob = outsbp.tile([QSH, DV + 1], dt.bfloat16, tag="ob",
                                 name=f"ob{b}")
                nc.vector.tensor_copy(ob[:], av[:])
                eng = nc.sync if bi == B - 1 else nc.gpsimd
                eng.dma_start(out=d_out[b], in_=ob[:])

    nc.compile()
    return nc


def _host_prep(queries, keys, values, vl, Wq, Wk, Wv):
    """Build the 8 per-core input maps (slicing / transposes / packing)."""
    import ml_dtypes
    bf16 = ml_dtypes.bfloat16

    queries = np.ascontiguousarray(np.asarray(queries, np.float32))
    keys = np.asarray(keys, np.float32)
    values = np.asarray(values, np.float32)
    Wq = np.asarray(Wq, np.float32)
    Wk = np.asarray(Wk, np.float32)
    Wv = np.asarray(Wv, np.float32)

    VL, VLP, NCH, BORD, WB, KW, cols, segs, CW = _geom(vl)

    shared = np.zeros((128, CW), np.float32)
    for e in range(2):
        shared[:, cols[f"wq{e}"]:cols[f"wq{e}"] + H] = \
            Wq[e * 128:(e + 1) * 128, :]
        shared[:, cols[f"wk{e}"]:cols[f"wk{e}"] + H] = \
            Wk[e * 128:(e + 1) * 128, :]
    for wi, (w0, wl, b) in enumerate(KW):
        wle = min(wl, VL[b] - w0)
        kT = keys[b, w0:w0 + wle, :].T.reshape(2, 128, wle)
        for e in range(2):
            c0 = cols[f"keysT_w{wi}_e{e}"]
            shared[:, c0:c0 + wle] = kT[e]
    for hc in range(2):
        shared[:, cols[f"wz{hc}"] + QSH - 1] = Wv[hc * 128:(hc + 1) * 128]
    for b in range(B):
        for ci in range(NCH[b]):
            c0 = ci * 128
            csz = min(128, VL[b] - c0)
            vcol = cols[f"vals{b}"] + ci * (DV + 1)
            shared[:csz, vcol:vcol + DV] = values[b, c0:c0 + csz, :]
            shared[:csz, vcol + DV] = 1.0
    shared[:, cols["ident"]:cols["ident"] + 128] = np.eye(128)

    in_maps = []
    for c in range(N_CORES):
        blob = shared.copy()
        qsl = queries[:, c * QSH:(c + 1) * QSH, :]  # [B, 16, QS]
        qT = np.ascontiguousarray(qsl.transpose(0, 2, 1))  # [B, QS, 16]
        for e in range(2):
            c0 = cols[f"qT{e}"]
            for b in range(B):
                blob[:, c0 + b * QSH:c0 + (b + 1) * QSH] = \
                    qT[b, e * 128:(e + 1) * 128, :]
        in_maps.append({"blob": blob.astype(bf16)})
    return in_maps


def _gather(results):
    out = np.empty((B, Q, DV), np.float32)
    for c in range(N_CORES):
        # device rows are in PERM order: row c holds q = PERM[c]; the
        # last column carries the softmax denominator (divide on host)
        av = np.asarray(results[c]["out"], np.float32)
        out[:, c * QSH + PERM, :] = av[:, :, :DV] / av[:, :, DV:DV + 1]
    return out


def kernel(queries, keys, values, valid_lens, Wq, Wk, Wv):
    from concourse.bass_utils import run_bass_kernel_spmd

    vl = tuple(int(x) for x in np.asarray(valid_lens).reshape(-1))
    assert len(vl) == B and all(1 <= v <= KMAX for v in vl)

    if vl not in _PROGRAM_CACHE:
        _PROGRAM_CACHE[vl] = _build_program(vl)
    nc = _PROGRAM_CACHE[vl]

    in_maps = _host_prep(queries, keys, values, vl, Wq, Wk, Wv)
    res = run_bass_kernel_spmd(nc, in_maps, list(range(N_CORES)))
    return _gather(res.results)


# revision 49
# speedup vs baseline: 1.1855x; 1.0127x over previous
"""Additive attention kernel for Trainium2, 8 NeuronCores (SPMD).

Reference computation (B=4, Q=128, K=1024, H=256, QS=KS=DV=256):
    q = queries @ Wq                    [B,Q,H]
    k = keys @ Wk                       [B,K,H]
    feats = tanh(q[:,:,None,:] + k[:,None,:,:])
    scores = feats @ Wv                 [B,Q,K]
    masked softmax over K (valid_lens), out = attn @ values   [B,Q,DV]

Sharding: Q is split across the 8 cores (16 q-rows per core, every core
processes all 4 batches) -- perfectly balanced, no collectives.  The
kernel is specialized at trace time on the runtime valid_lens values, so
only valid key positions are ever computed.

Per-core device program (H on partitions, 2 chunks of 128).  The ACT
(scalar) engine's tanh stream is the roofline (1 elem/cycle/lane); the
whole program is organized to keep it saturated:

  DMA: the packed bf16 blob is split into ~10 column segments spread over
    the three DMA-capable queues (sync HWDGE, scalar HWDGE, gpsimd SWDGE),
    ordered by consumer need-time so the first batch's projection inputs
    land ~1.5us after kickoff and everything else streams in under the
    tanh phases.  Batches are processed smallest-first (fast pipeline
    ramp); the last batch is chosen small to shorten the drain tail.
  Projections (PE, bf16, fp32 PSUM accumulate over the 2 e-chunks):
    qprojT[hc] = Wq_hc.T @ qT [128, B*16]; kprojT per batch [128, VLP].
    kproj for batch b+1 is emitted in the middle of batch b's sweep so PE
    fills its idle slots; all PSUM->SBUF kp casts live on the DVE (the
    ACT engine runs nothing but tanh+exp).
  Scores, per (b, hc, slab of q's): DVE tensor_scalar_add (kprojT + qp
    column, 4x mode) into a bf16 slab, ONE ACT Tanh per slab, then per q
    a matmul whose lhsT is a width-32 slice of a sliding one-hot Wv tile,
    round-robined over the four PE column groups (tile_position=(0,32*s),
    s=q%4, one-hot at row q//4).  The score PSUM banks are zeroed by DVE
    memsets issued up-front during the DMA wait (zero marginal cost).
    Only the very first matmul per PSUM bank uses start=True (the start
    flag clears has_written for the WHOLE bank; concurrent per-stripe
    starts race).
  Softmax + AV, per batch: exp straight off the PSUM stripes (ACT, bf16
    out; zero rows give exp=1, never read; |scores| <= sum|Wv| ~ 13 so
    no max shift), PE bf16 transpose per 128-chunk, DVE compaction of
    the 16 live columns, AV matmul against the ones-augmented bf16
    values (the ones column accumulates the softmax denominator), then
    the [16, DV+1] AV accumulator ships straight from PSUM to DRAM; the
    softmax division and the q-permutation introduced by the striping
    (q -> 4*(q%4) + q//4) are undone on the host after gather.
"""

import numpy as np

B, Q, KMAX, H = 4, 128, 1024, 256
QS, KS, DV = 256, 256, 256
N_CORES = 8
QSH = Q // N_CORES  # 16 q rows per core

# q index permutation induced by the 4-way PSUM striping (self-inverse)
PERM = np.array([4 * (c % 4) + c // 4 for c in range(QSH)])

_PROGRAM_CACHE: dict = {}


def _even(x: int) -> int:
    return x + (x & 1)


def _windows(total: int, step: int):
    out = []
    s = 0
    while s < total:
        out.append((s, min(step, total - s)))
        s += step
    return out


def _batch_order(VL):
    """Second-largest batch first (its long, ACT-bound tanh slabs absorb
    the DVE ramp: qp copies + next batch's kp casts), then the largest,
    then descending, smallest last (short drain tail)."""
    order = sorted(range(B), key=lambda b: -VL[b])
    return [order[1], order[0]] + order[2:]


def _slabs(b, b_idx, hc, n_batches, VLP):
    """Slab sizes (q's per tanh) for batch b at position b_idx."""
    if b_idx == 0 and hc == 0:
        return [4, 4, 8]  # ramp-in: first tanh fires after only 4 adds
    if b_idx == n_batches - 1 and hc == 1:
        return [8, 4, 4]  # fine-grained drain
    if b_idx == 1 and hc == 0:
        # batch-to-batch entry: the first tanh here only needs 4 adds
        # pre-buffered during the previous batch's final slab
        return [4, 4, 8]
    if hc == 1 and 0 < b_idx < n_batches - 1:
        # mid-stream second chunk: the DVE conveyor is far enough ahead
        # that one big ACTIVATE (half the fill overhead) is safe
        return [16]
    return [8, 8]


def _geom(vl):
    """Shared geometry: valid lens, paddings, blob column offsets.

    Blob columns are laid out in DMA-segment order: each segment is a
    contiguous column range pushed as one dma_start on its queue.
    """
    VL = list(vl)
    VLP = [_even(v) for v in VL]
    NCH = [(v + 127) // 128 for v in VL]
    BORD = _batch_order(VL)

    WB = {b: _windows(VLP[b], 512) for b in range(B)}
    KW = []  # (global window idx) -> (w0-in-batch, wl, batch)
    for b in range(B):
        for (u0, ul) in WB[b]:
            KW.append((u0, ul, b))

    cols = {}
    segs = {"sync": [], "scalar": [], "gpsimd": []}
    c = 0

    def seg(queue, names_widths):
        nonlocal c
        c0 = c
        for name, w in names_widths:
            cols[name] = c
            c += w
        segs[queue].append((c0, c))

    def keys_seg(b):
        out = []
        for wi, (_w0, wl, wb) in enumerate(KW):
            if wb == b:
                for e in range(2):
                    out.append((f"keysT_w{wi}_e{e}", wl))
        return out

    def vals_names(b):
        return [(f"vals{b}", NCH[b] * (DV + 1))]

    # sync queue: projections first, then values in batch order
    seg("sync", [("wq0", H), ("wq1", H), ("qT0", B * QSH), ("qT1", B * QSH)])
    seg("sync", vals_names(BORD[0]) + [("ident", 128)])
    seg("sync", vals_names(BORD[2]) + vals_names(BORD[3]))
    # scalar queue: first batch's kproj inputs, then second batch's keys
    seg("scalar", [("wk0", H), ("wk1", H)] + keys_seg(BORD[0]))
    seg("scalar", keys_seg(BORD[1]))
    # gpsimd queue: one-hot Wv tiles, remaining keys, 2nd batch's values
    seg("gpsimd", [("wz0", QSH + 31), ("wz1", QSH + 31)])
    seg("gpsimd", keys_seg(BORD[2]))
    seg("gpsimd", keys_seg(BORD[3]))
    seg("gpsimd", vals_names(BORD[1]))
    cols["zpad"] = c
    c += 16
    CW = c
    return VL, VLP, NCH, BORD, WB, KW, cols, segs, CW


def _build_program(vl: tuple):
    import concourse.bacc as bacc
    import concourse.mybir as mybir
    import concourse.tile as tile

    dt = mybir.dt
    AF = mybir.ActivationFunctionType
    VL, VLP, NCH, BORD, WB, KW, cols, segs, CW = _geom(vl)

    nc = bacc.Bacc("TRN2", target_bir_lowering=False, debug=False,
                   num_devices=N_CORES)

    d_blob = nc.dram_tensor("blob", [128, CW], dt.bfloat16,
                            kind="ExternalInput")
    d_out = nc.dram_tensor("out", [B, QSH, DV + 1], dt.bfloat16,
                           kind="ExternalOutput")

    with tile.TileContext(nc) as tc:
        with (
            tc.tile_pool(name="const", bufs=1) as constp,
            tc.tile_pool(name="kp", bufs=1) as kpp,
            tc.tile_pool(name="qp", bufs=2) as qpp,
            tc.tile_pool(name="pre", bufs=5) as prep,
            tc.tile_pool(name="feats", bufs=5) as featsp,
            tc.tile_pool(name="scsb", bufs=2) as scsbp,
            tc.tile_pool(name="expt", bufs=4) as exptp,
            tc.tile_pool(name="outsb", bufs=2) as outsbp,
            tc.tile_pool(name="pswork", bufs=2, space="PSUM") as pswork,
            tc.tile_pool(name="pssc", bufs=5, space="PSUM") as pssc,
            tc.tile_pool(name="psav", bufs=1, space="PSUM") as psav,
        ):
            blob = constp.tile([128, CW], dt.bfloat16, tag="blob")
            eng_of = {"sync": nc.sync, "scalar": nc.scalar,
                      "gpsimd": nc.gpsimd}
            for qname in ("sync", "scalar", "gpsimd"):
                for (c0, c1) in segs[qname]:
                    eng_of[qname].dma_start(out=blob[:, c0:c1],
                                            in_=d_blob[:, c0:c1])

            def bl(name, width):
                c0 = cols[name]
                return blob[:, c0:c0 + width]

            # ---- q projection ----
            qp = [None, None]  # [hc] -> [128, B*QSH] f32 in SBUF
            for hc in range(2):
                hs = slice(hc * 128, hc * 128 + 128)
                ps = pswork.tile([128, B * QSH], dt.float32, tag="w",
                                 name=f"psq{hc}")
                nc.tensor.matmul(ps[:], bl("wq0", H)[:, hs],
                                 bl("qT0", B * QSH), start=True, stop=False)
                nc.tensor.matmul(ps[:], bl("wq1", H)[:, hs],
                                 bl("qT1", B * QSH), start=False, stop=True)
                qp[hc] = qpp.tile([128, B * QSH], dt.float32, tag="qp",
                                  name=f"qp{hc}")
                nc.vector.tensor_copy(qp[hc][:], ps[:])

            # ---- k projection (per batch, emitted lazily) ----
            # PE matmuls and DVE casts are emitted separately: the PE has
            # slack early (it idles until the first tanh lands), but a
            # cast emitted too early blocks the DVE's add stream behind
            # the next batch's keysT DMA.
            kp = {}       # b -> [hc] -> [128, VLP[b]] bf16 in SBUF
            kp_ps = {}    # b -> list of (hc, w0, wl, psum tile)

            def emit_kproj_mm(b):
                kp[b] = [kpp.tile([128, VLP[b]], dt.bfloat16,
                                  tag=f"kp{b}_{hc}", name=f"kp{b}_{hc}")
                         for hc in range(2)]
                kp_ps[b] = []
                for wi, (w0, wl, wb) in enumerate(KW):
                    if wb != b:
                        continue
                    for hc in range(2):
                        hs = slice(hc * 128, hc * 128 + 128)
                        ps2 = pswork.tile([128, wl], dt.float32, tag="w",
                                          name=f"psk{hc}_{b}_{w0}")
                        nc.tensor.matmul(ps2[:], bl("wk0", H)[:, hs],
                                         bl(f"keysT_w{wi}_e0", wl),
                                         start=True, stop=False)
                        nc.tensor.matmul(ps2[:], bl("wk1", H)[:, hs],
                                         bl(f"keysT_w{wi}_e1", wl),
                                         start=False, stop=True)
                        kp_ps[b].append((hc, w0, wl, ps2))

            def emit_kproj_cast(b, hcs=(0, 1)):
                for hc, w0, wl, ps2 in kp_ps[b]:
                    if hc in hcs:
                        nc.vector.tensor_copy(
                            kp[b][hc][:, w0:w0 + wl], ps2[:])

            emit_kproj_mm(BORD[0])
            emit_kproj_cast(BORD[0])
            emit_kproj_mm(BORD[1])

            # score PSUM banks: allocate + zero all of them up-front, on
            # the DVE, while it would otherwise idle waiting for DMA
            sc_tiles = {}
            for b in BORD:
                for wi in range(len(WB[b])):
                    t = pssc.tile([128, 512], dt.float32, tag="sc",
                                  name=f"sc{b}_{wi}")
                    nc.vector.memset(t[:], 0.0)
                    sc_tiles[b, wi] = t

            for bi, b in enumerate(BORD):
                vlb, vlpb = VL[b], VLP[b]
                # ---- scores sweep for batch b (4-way col-packed) ----
                sc_ps = [sc_tiles[b, wi] for wi in range(len(WB[b]))]
                for hc in range(2):
                    q0 = 0
                    for gi, gsz in enumerate(_slabs(b, bi, hc, B, VLP)):
                        pre_t = prep.tile([128, gsz * vlpb], dt.bfloat16,
                                          tag="pre", name=f"pre{b}{hc}{gi}")
                        for j in range(gsz):
                            qq = q0 + j
                            nc.vector.tensor_scalar_add(
                                pre_t[:, j * vlpb:(j + 1) * vlpb],
                                kp[b][hc][:],
                                qp[hc][:, b * QSH + qq:b * QSH + qq + 1])
                        feats_t = featsp.tile([128, gsz * vlpb], dt.bfloat16,
                                              tag="feats",
                                              name=f"ft{b}{hc}{gi}")
                        nc.scalar.activation(feats_t[:], pre_t[:], AF.Tanh)
                        if bi == 0 and gi == len(
                                _slabs(b, bi, hc, B, VLP)) - 1:
                            # DVE slack opens before the last slab of each
                            # hc phase: pull the next batch's kp casts in,
                            # half per phase so neither phase bubbles
                            emit_kproj_cast(BORD[1], hcs=(hc,))
                        for j in range(gsz):
                            qq = q0 + j
                            s, r = qq % 4, qq // 4
                            # one-hot column r within a width-32 window
                            lhsT = bl(f"wz{hc}", QSH + 31)[
                                :, QSH - 1 - r: QSH + 31 - r]
                            first = (hc == 0 and qq == 0)
                            last = (hc == 1 and qq == QSH - 1)
                            for wi, (w0, wl) in enumerate(WB[b]):
                                nc.tensor.matmul(
                                    sc_ps[wi][32 * s:32 * s + 32, 0:wl],
                                    lhsT,
                                    feats_t[:, j * vlpb + w0:
                                            j * vlpb + w0 + wl],
                                    start=first, stop=last,
                                    tile_position=(0, 32 * s),
                                    skip_group_check=True)
                        q0 += gsz
                    if hc == 0:
                        # keep the pipeline fed: batch bi+2's kproj MMs
                        # and batch bi+1's deferred kp casts go out here
                        if bi >= 1 and bi + 1 < B:
                            emit_kproj_cast(BORD[bi + 1])
                        if bi + 2 < B:
                            emit_kproj_mm(BORD[bi + 2])

                # ---- scores -> softmax -> AV for batch b ----
                # exp straight off the PSUM stripes (garbage-free rows are
                # exactly 0 -> exp=1, never read), then bf16 transpose
                exp_sb = scsbp.tile([128, vlb], dt.bfloat16, tag="scsb",
                                    name=f"scsb{b}")
                for wi, (w0, wl) in enumerate(WB[b]):
                    wle = min(wl, vlb - w0)  # drop the pad column
                    nc.scalar.activation(exp_sb[:, w0:w0 + wle],
                                         sc_ps[wi][:, 0:wle], AF.Exp)

                av = psav.tile([QSH, DV + 1], dt.float32, tag="av",
                               name=f"av{b}")
                for ci in range(NCH[b]):
                    c0 = ci * 128
                    csz = min(128, vlb - c0)
                    trp = pswork.tile([csz, 128], dt.bfloat16, tag="w",
                                      name=f"tr{b}_{ci}")
                    nc.tensor.transpose(trp[:], exp_sb[:, c0:c0 + csz],
                                        bl("ident", 128))
                    ex = exptp.tile([csz, QSH], dt.bfloat16, tag="ex",
                                    name=f"ex{b}_{ci}")
                    nc.vector.tensor_copy(
                        ex[:].rearrange("p (s r) -> p s r", s=4),
                        trp[:].rearrange("p (s x r) -> p s x r",
                                         s=4, x=8)[:, :, 0, 0:4])
                    vcol = cols[f"vals{b}"] + ci * (DV + 1)
                    nc.tensor.matmul(av[:], ex[:],
                                     blob[0:csz, vcol:vcol + DV + 1],
                                     start=(ci == 0),
                                     stop=(ci == NCH[b] - 1))

                # ship numerator + denominator; the host divides after
                # gather.  Mid-stream batches go on the (idle) gpsimd
                # queue; the final batch uses the sync HWDGE queue, whose
                # hardware descriptor generation shaves ~1us off the
                # critical out-DMA -> final-barrier path.
                ob = outsbp.tile([QSH, DV + 1], dt.bfloat16, tag="ob",
                                 name=f"ob{b}")
                nc.vector.tensor_copy(ob[:], av[:])
                eng = nc.sync if bi == B - 1 else nc.gpsimd
                eng.dma_start(out=d_out[b], in_=ob[:])

    nc.compile()
    return nc


def _host_prep(queries, keys, values, vl, Wq, Wk, Wv):
    """Build the 8 per-core input maps (slicing / transposes / packing)."""
    import ml_dtypes
    bf16 = ml_dtypes.bfloat16

    queries = np.ascontiguousarray(np.asarray(queries, np.float32))
    keys = np.asarray(keys, np.float32)
    values = np.asarray(values, np.float32)
    Wq = np.asarray(Wq, np.float32)
    Wk = np.asarray(Wk, np.float32)
    Wv = np.asarray(Wv, np.float32)

    VL, VLP, NCH, BORD, WB, KW, cols, segs, CW = _geom(vl)

    shared = np.zeros((128, CW), np.float32)
    for e in range(2):
        shared[:, cols[f"wq{e}"]:cols[f"wq{e}"] + H] = \
            Wq[e * 128:(e + 1) * 128, :]
        shared[:, cols[f"wk{e}"]:cols[f"wk{e}"] + H] = \
            Wk[e * 128:(e + 1) * 128, :]
    for wi, (w0, wl, b) in enumerate(KW):
        wle = min(wl, VL[b] - w0)
        kT = keys[b, w0:w0 + wle, :].T.reshape(2, 128, wle)
        for e in range(2):
            c0 = cols[f"keysT_w{wi}_e{e}"]
            shared[:, c0:c0 + wle] = kT[e]
    for hc in range(2):
        shared[:, cols[f"wz{hc}"] + QSH - 1] = Wv[hc * 128:(hc + 1) * 128]
    for b in range(B):
        for ci in range(NCH[b]):
            c0 = ci * 128
            csz = min(128, VL[b] - c0)
            vcol = cols[f"vals{b}"] + ci * (DV + 1)
            shared[:csz, vcol:vcol + DV] = values[b, c0:c0 + csz, :]
            shared[:csz, vcol + DV] = 1.0
    shared[:, cols["ident"]:cols["ident"] + 128] = np.eye(128)

    in_maps = []
    for c in range(N_CORES):
        blob = shared.copy()
        qsl = queries[:, c * QSH:(c + 1) * QSH, :]  # [B, 16, QS]
        qT = np.ascontiguousarray(qsl.transpose(0, 2, 1))  # [B, QS, 16]
        for e in range(2):
            c0 = cols[f"qT{e}"]
            for b in range(B):
                blob[:, c0 + b * QSH:c0 + (b + 1) * QSH] = \
                    qT[b, e * 128:(e + 1) * 128, :]
        in_maps.append({"blob": blob.astype(bf16)})
    return in_maps


def _gather(results):
    out = np.empty((B, Q, DV), np.float32)
    for c in range(N_CORES):
        # device rows are in PERM order: row c holds q = PERM[c]; the
        # last column carries the softmax denominator (divide on host)
        av = np.asarray(results[c]["out"], np.float32)
        out[:, c * QSH + PERM, :] = av[:, :, :DV] / av[:, :, DV:DV + 1]
    return out


def kernel(queries, keys, values, valid_lens, Wq, Wk, Wv):
    from concourse.bass_utils import run_bass_kernel_spmd

    vl = tuple(int(x) for x in np.asarray(valid_lens).reshape(-1))
    assert len(vl) == B and all(1 <= v <= KMAX for v in vl)

    if vl not in _PROGRAM_CACHE:
        _PROGRAM_CACHE[vl] = _build_program(vl)
    nc = _PROGRAM_CACHE[vl]

    in_maps = _host_prep(queries, keys, values, vl, Wq, Wk, Wv)
    res = run_bass_kernel_spmd(nc, in_maps, list(range(N_CORES)))
    return _gather(res.results)
